# revision 48
# baseline (speedup 1.0000x reference)
"""Multi-head causal attention (B=4, T=2048, D=1024, H=16) on 8 Trainium2
NeuronCores.

Sharding: core c -> (batch = c//2, head-group g = c%2, 8 heads each).
Each core: QKV projection for its batch/head-group, causal attention,
then a per-head-pair 2-way AllGather of the attention output with its
pair core (same batch, other head-group), followed by a fully LOCAL
output projection over this core's 512 output channels (full 1024-
channel contraction) written straight to out. Host reassembles with a
transpose+concat only.

On-chip orientation is "transposed" throughout (channels on partitions,
tokens on the free dim):
  xT   (D, T)    bf16, pre-transposed on HOST
  qkT  (1024, T) = wqk.T @ xT  (q rows pre-scaled by 1/sqrt(Dh) on host)
  v    (T, 512)  natural, with an extra all-ones column per head slot
  sT   (k, q)    = K_tile @ qT  -> exp on ScalarE -> es (bf16)
  outT (65, q)   = [v|1].T @ es  (fp32 PSUM; row 64 = softmax denom)
  oh   (512, q)  normalized attention out, exchanged with the pair core
  final (512own, q) = wp_all.T @ ohAG + bias  -> DMA to out

Key performance structure:
- Scores matmuls have a 64-deep contraction (Dh=64), so the PE array is
  row-tiled 64x128: head h0 (its q/k channels on SBUF partitions 0-63)
  runs on PE tile (0,0) while h1 (partitions 64-127) runs CONCURRENTLY
  on tile (64,0) -> ~2x scores throughput. For the pair to actually
  overlap, both matmuls must become ready at the SAME event, so each
  exp covers one k-tile for BOTH heads ([P, 2, 512] scores tile), and
  consecutive k-tiles alternate between two single-buffered PSUM pools
  so exp(kt) and the next scores(kt+1) pipeline with zero stall.
- The QKV projection is produced in 512-token chunks and chunk qc+1 is
  woven INTO the step loop of q-macro qc (one group every other k-tile)
  so the PE stays dense while ScalarE runs exp.
- AV matmuls (full 128 contraction) run one k-tile behind scores/exp.
- Staged startup DMAs: wv + xT chunk 0 land first, alone, on two
  queues; the wqk Q-half / K-half loads (and later xT chunks / wp) are
  released by tiny WAR-gate copies only once the prior wave lands, so
  each wave gets full bandwidth and the V -> Q -> K projection chain
  starts as early as the data allows.
- Causal mask: strictly-upper triangular 128x128 multiply (VectorE) on
  boundary tiles only; future k-tiles are never computed.
- Tested dead end (do not retry): emulating CollectivePermute with a
  masked ReduceScatter (slab staged twice scaled by a host per-core
  mask [g, 1-g]; RS-add delivers the pair's slab rank-agnostically;
  own-rank out-projection halves read local SBUF). Numerically exact,
  but ReduceScatter on the CC cores ran ~50us SLOWER end-to-end than
  AllGather here (2x input bytes through an fp32 add-reduce), swamping
  the dependency-decoupling win. AllGather with rank-ordered wp (as
  below) is the faster exchange on this hardware.
- Two 2-way slab-pair AllGathers per q-macro, issued as each pair of
  head-pair slabs completes (per-slab gathers for the last q-macro's
  hp2/hp3). The last q-macro's out-projection accumulates slab-by-slab
  in PSUM banks freed by the finished attention, so the tail is one
  64KB-per-direction hop + 8 matmuls instead of a full gather +
  projection. Gather-output copy-backs ride the gpsimd DMA queue so
  slab staging on the sync queue never head-of-line blocks them.
"""

import numpy as np
import ml_dtypes

import concourse.bass as bass
from concourse import bacc
import concourse.mybir as mybir
import concourse.tile as tile
from concourse.bass_utils import run_bass_kernel_spmd
from concourse.masks import make_upper_triangular

B, T, D = 4, 2048, 1024
H_TOT, DH = 16, 64
HL = 8          # heads per core
P = 128
ND = D // P     # 8 d-tiles
NT = T // P     # 16 token tiles
NQ = T // 512   # 4 q-macros
F32 = mybir.dt.float32
BF16 = mybir.dt.bfloat16
AF = mybir.ActivationFunctionType
NP_BF16 = ml_dtypes.bfloat16

REPLICA_GROUPS = [[0, 1], [2, 3], [4, 5], [6, 7]]


def build_bass():
    nc = bacc.Bacc(None, target_bir_lowering=False, num_devices=8)

    # all inputs pre-permuted on host into SBUF layout (partition-major)
    # so every input DMA is a contiguous read
    xT = nc.dram_tensor("xT", [4, P, ND, 512], BF16, kind="ExternalInput")
    wqk = nc.dram_tensor("wqk", [P, 2, ND, 512], BF16, kind="ExternalInput")
    wv = nc.dram_tensor("wv", [P, ND, 512], BF16, kind="ExternalInput")
    # full 1024 contraction rows x this core's 512 output cols
    wp = nc.dram_tensor("wp", [P, 2, 4, 512], BF16, kind="ExternalInput")
    bias = nc.dram_tensor("bias", [P, 4], F32, kind="ExternalInput")
    out = nc.dram_tensor("out", [4, P, 4, 512], BF16, kind="ExternalOutput")

    with tile.TileContext(nc, num_cores=8) as tc:
        with (
            tc.tile_pool(name="const", bufs=1) as const_pool,
            tc.tile_pool(name="dram", bufs=1, space="DRAM") as dram_pool,
            tc.tile_pool(name="persist", bufs=1) as persist,
            tc.tile_pool(name="wp_pool", bufs=1) as wp_pool,
            tc.tile_pool(name="es_pool", bufs=10) as es_pool,
            tc.tile_pool(name="oh_pool", bufs=3) as oh_pool,
            tc.tile_pool(name="og_pool", bufs=2) as og_pool,
            tc.tile_pool(name="ohu_pool", bufs=3) as ohu_pool,
            tc.tile_pool(name="cs_pool", bufs=2) as cs_pool,
            tc.tile_pool(name="rb_pool", bufs=3) as rb_pool,
            tc.tile_pool(name="po_pool", bufs=3) as po_pool,
            # two single-buffered score pools so exp(kt) and the next
            # scores(kt+1) ping-pong without a shared-tile WAR stall
            tc.tile_pool(name="ps_sa", bufs=1, space="PSUM") as ps_sa,
            tc.tile_pool(name="ps_sb", bufs=1, space="PSUM") as ps_sb,
            tc.tile_pool(name="ps_av", bufs=2, space="PSUM") as ps_av,
            tc.tile_pool(name="ps_mm", bufs=2, space="PSUM") as ps_mm,
        ):
            tri = const_pool.tile([P, P], BF16)
            bias_sb = const_pool.tile([P, 4], F32)
            ones_row = const_pool.tile([1, P], F32)

            qkT = persist.tile([P, ND, T], BF16, name="qkT")
            v1 = persist.tile([P, NT, HL, DH + 1], BF16, name="v1")
            ones_sb = const_pool.tile([P, NT, HL, 1], F32)
            # wp2[p, rank, hp, c] = w_proj[rank*512 + hp*128 + p, mycols[c]]
            wp2_sb = wp_pool.tile([P, 2, 4, 512], BF16)

            score_pools = [ps_sa, ps_sb]

            def attention(qm, weave):
                """Attention for q-macro qm; pulls from `weave` (an iterator
                of thunks emitting projection matmul groups) every other
                k-tile. Slab pairs are staged to DRAM and exchanged with
                the pair core as they complete (per-slab for the last
                q-macro's tail)."""
                nkt = 4 * qm + 4
                nb = nkt - 4
                fine_tail = (qm == 3)
                oh_sb = oh_pool.tile([P, 4, 512], BF16, name="oh_sb")
                parts = []

                for hp in range(4):
                    heads = (2 * hp, 2 * hp + 1)
                    out_ps = {}
                    av_issued = {}
                    for h in heads:
                        out_ps[h] = ps_av.tile([DH + 1, 512], F32,
                                               name=f"out_ps{h % 2}",
                                               tag="ps_av")
                        av_issued[h] = 0

                    def av_mm(h, kt, src):
                        nc.tensor.matmul(
                            out_ps[h][:, max(0, P * kt - 512 * qm):],
                            lhsT=v1[:, kt, h, :],
                            rhs=src,
                            start=(av_issued[h] == 0),
                            stop=(av_issued[h] == nkt - 1),
                            skip_group_check=True)
                        av_issued[h] += 1

                    def scores_mm(h, kt, dst, qoff):
                        # Dh=64 contraction -> 64x128 row tile: h0 on PE
                        # tile (0,0), h1 on (64,0). Both are gated on the
                        # same exp, issued back-to-back -> they overlap.
                        hi = (h % 2) * DH
                        nc.tensor.matmul(
                            dst,
                            lhsT=qkT[hi:hi + DH, 4 + h // 2,
                                     kt * P:(kt + 1) * P],
                            rhs=qkT[hi:hi + DH, h // 2,
                                    qm * 512 + qoff:(qm + 1) * 512],
                            start=True, stop=True,
                            tile_position=(hi, 0))

                    # software pipeline: AVs one k-tile behind scores/exp
                    pend = []

                    def flush():
                        for h_, kt_, src_ in pend:
                            av_mm(h_, kt_, src_)
                        pend.clear()

                    def pull_weave():
                        if weave is not None:
                            thunk = next(weave, None)
                            if thunk is not None:
                                thunk()

                    for kt in range(nb):        # non-boundary k-tiles
                        st = score_pools[kt % 2].tile(
                            [P, 2, 512], F32, name="st", tag="st")
                        for h in heads:
                            scores_mm(h, kt, st[:, h % 2, :], 0)
                        ese = es_pool.tile([P, 2, 512], BF16, name="es",
                                           tag="es")
                        nc.scalar.activation(out=ese, in_=st, func=AF.Exp)
                        flush()
                        for h in heads:
                            pend.append((h, kt, ese[:, h % 2, :]))
                        if kt % (4 if qm == 2 else 2) == 1:
                            pull_weave()
                    # boundary k-tiles, masked. ktA = nb+2+bp runs in pool
                    # a, ktB = nb+bp in pool b, same kt-alternating rhythm.
                    for bp in range(2):
                        ktA, ktB = nb + 2 + bp, nb + bp
                        qoffA = P * ktA - 512 * qm
                        qoffB = P * ktB - 512 * qm
                        for kt, qoff, pool in ((ktA, qoffA, score_pools[0]),
                                               (ktB, qoffB, score_pools[1])):
                            st = pool.tile([P, 2, 512], F32, name="st",
                                           tag="st")
                            for h in heads:
                                scores_mm(h, kt, st[:, h % 2, qoff:], qoff)
                            ese = es_pool.tile([P, 2, 512], BF16, name="es",
                                               tag="es")
                            nc.scalar.activation(
                                out=ese[:, :, qoff:], in_=st[:, :, qoff:],
                                func=AF.Exp)
                            for h in heads:
                                nc.vector.tensor_mul(
                                    ese[:, h % 2, qoff:qoff + P],
                                    ese[:, h % 2, qoff:qoff + P], tri)
                            flush()
                            for h in heads:
                                pend.append((h, kt, ese[:, h % 2, qoff:]))
                        pull_weave()
                    flush()

                    # evacuate PSUM accumulators, then normalize
                    ohu = ohu_pool.tile([P, 512], F32, name="ohu")
                    cs = cs_pool.tile([P, 2, 512], F32, name="cs")
                    for i, h in enumerate(heads):
                        hi = (h % 2) * DH
                        nc.vector.tensor_copy(
                            out=ohu[hi:hi + DH, :], in_=out_ps[h][0:DH, :])
                        nc.vector.tensor_copy(
                            out=cs[0:1, i, :], in_=out_ps[h][DH:DH + 1, :])
                    # broadcast raw denominators, then one fast approx
                    # reciprocal across all 128 lanes
                    if fine_tail and hp >= 2:
                        rr = rb_pool.tile([P, 2, 512], F32, name="rbr")
                        nc.vector.reciprocal_approx_fast(
                            out=rr[0:1, :, :], in_=cs[0:1, :, :])
                        rbp = {}
                        for i in range(2):
                            rbp[i] = ps_mm.tile([P, 512], F32, name="ps_f",
                                                tag="ps_mm")
                            nc.tensor.matmul(rbp[i], lhsT=ones_row,
                                             rhs=rr[0:1, i, :],
                                             start=True, stop=True)
                        for i, h in enumerate(heads):
                            hi = (h % 2) * DH
                            nc.vector.tensor_mul(
                                oh_sb[hi:hi + DH, hp, :],
                                ohu[hi:hi + DH, :], rbp[i][hi:hi + DH, :])
                    else:
                        rbr = rb_pool.tile([P, 2, 512], F32, name="rbr")
                        nc.gpsimd.partition_broadcast(rbr[:, 0, :],
                                                      cs[0:1, 0, :])
                        nc.gpsimd.partition_broadcast(rbr[:, 1, :],
                                                      cs[0:1, 1, :])
                        rb = rb_pool.tile([P, 2, 512], F32, name="rb")
                        nc.vector.reciprocal_approx_fast(out=rb, in_=rbr)
                        for i, h in enumerate(heads):
                            hi = (h % 2) * DH
                            nc.vector.tensor_mul(
                                oh_sb[hi:hi + DH, hp, :],
                                ohu[hi:hi + DH, :], rb[hi:hi + DH, i, :])

                    # exchange slabs as they complete. Slab-PAIRS are
                    # staged with one 2KB-row DMA (2x the packet size of
                    # per-slab staging); the last q-macro instead gathers
                    # hp2 and hp3 individually so the tail's last
                    # dependency is a single 64KB-per-direction hop.
                    def exchange(lo, hi, tag):
                        n = hi - lo
                        ohd = dram_pool.tile([P, n * 512], BF16,
                                             name=f"ohd{tag}_{qm}",
                                             tag=f"ohd{tag}_{qm}")
                        if fine_tail and n == 1:
                            nc.sync.dma_start(out=ohd[0:64, :],
                                              in_=oh_sb[0:64, lo:hi, :])
                            nc.scalar.dma_start(out=ohd[64:P, :],
                                                in_=oh_sb[64:P, lo:hi, :])
                        else:
                            nc.sync.dma_start(out=ohd[:, :],
                                              in_=oh_sb[:, lo:hi, :])
                        ohg = dram_pool.tile([2 * P, n * 512], BF16,
                                             name=f"ohg{tag}_{qm}",
                                             tag=f"ohg{tag}_{qm}")
                        nc.gpsimd.collective_compute(
                            "AllGather", mybir.AluOpType.bypass,
                            replica_groups=REPLICA_GROUPS,
                            ins=[ohd[:, :]], outs=[ohg[:, :]])
                        parts.append((ohg, lo, hi))
                    if hp == 1:
                        exchange(0, 2, "A")
                    elif hp == 2 and fine_tail:
                        exchange(2, 3, "B")
                    elif hp == 3:
                        if fine_tail:
                            exchange(3, 4, "C")
                        else:
                            exchange(2, 4, "B")

                    pull_weave()
                    if qm >= 2:
                        pull_weave()

                return parts

            def back_copies(parts, dst, eng=None):
                """DMA AllGathered slabs (DRAM) back into SBUF dst
                [P, rank, hp, 512]. Issued on the gpsimd DMA queue so
                they are not head-of-line blocked behind slab staging on
                the sync queue."""
                eng = eng or nc.gpsimd
                for ohg, lo, hi in parts:
                    ns = hi - lo
                    for r in range(2):
                        eng.dma_start(
                            out=dst[:, r, lo:hi, :],
                            in_=ohg[r * P:(r + 1) * P, :])

            def oc_group(qm, ohg_sb, oc):
                """This core's output cols [128*oc, 128*oc+128): full
                1024-channel contraction over both ranks' slabs."""
                ps = ps_mm.tile([P, 512], F32, name="ps_f", tag="ps_mm")
                first = True
                for rank in range(2):
                    for hp in range(4):
                        nc.tensor.matmul(
                            ps,
                            lhsT=wp2_sb[:, rank, hp, oc * P:(oc + 1) * P],
                            rhs=ohg_sb[:, rank, hp, :],
                            start=first, stop=(rank == 1 and hp == 3))
                        first = False
                po = po_pool.tile([P, 512], BF16, name="po")
                nc.vector.tensor_scalar_add(
                    out=po, in0=ps, scalar1=bias_sb[:, oc:oc + 1])
                nc.sync.dma_start(out=out.ap()[qm][:, oc, :], in_=po)

            def out_proj_groups(qm, parts):
                """First thunk copies the AllGathered slabs back to SBUF;
                the rest project 128 output cols each."""
                ohg_sb = og_pool.tile([P, 2, 4, 512], BF16, name="ohg_sb")

                def back():
                    back_copies(parts, ohg_sb)
                yield back
                for oc in range(4):
                    yield (lambda qm=qm, og=ohg_sb, oc=oc:
                           oc_group(qm, og, oc))

            def final_out_proj(qm, parts):
                """qm3 tail: accumulate the out-projection slab-group by
                slab-group as the two half-gathers land, using PSUM banks
                the finished attention no longer needs."""
                og3 = og_pool.tile([P, 2, 4, 512], BF16, name="ohg_sb")
                # partition-halved copy-backs across two rings (keeps the
                # 1KB packet size, halves the serial hop on the tail)
                for ohg, lo, hi in parts:
                    for r in range(2):
                        nc.scalar.dma_start(
                            out=og3[0:64, r, lo:hi, :],
                            in_=ohg[r * P:r * P + 64, :])
                        nc.sync.dma_start(
                            out=og3[64:P, r, lo:hi, :],
                            in_=ohg[r * P + 64:(r + 1) * P, :])
                acc = [pool.tile([P, 2, 512], F32, name="st", tag="st")
                       for pool in score_pools]

                def acc_ap(oc):
                    return acc[oc // 2][:, oc % 2, :]

                def acc_mms(hp_range):
                    for hp in hp_range:
                        for r in range(2):
                            for oc in range(4):
                                nc.tensor.matmul(
                                    acc_ap(oc),
                                    lhsT=wp2_sb[:, r, hp,
                                                oc * P:(oc + 1) * P],
                                    rhs=og3[:, r, hp, :],
                                    start=(hp == 0 and r == 0),
                                    stop=(hp == 3 and r == 1),
                                    skip_group_check=True)
                # slabs hp0-2 are gathered well before the attention ends:
                # accumulate them immediately, THEN keep the PE warm while
                # the last 64KB gather + copy-back drains, then finish hp3
                acc_mms(range(3))
                warm_ps = ps_mm.tile([P, 512], F32, name="ps_f",
                                     tag="ps_mm")
                for w in range(56):
                    nc.tensor.matmul(
                        warm_ps,
                        lhsT=wp2_sb[:, 0, 0, 0:P],
                        rhs=wp2_sb[:, 0, w % 4, 0:512],
                        start=(w == 0), stop=(w == 55))
                warm_sink = po_pool.tile([P, 8], F32, name="warm_sink")
                nc.vector.tensor_copy(out=warm_sink, in_=warm_ps[:, 0:8])
                acc_mms(range(3, 4))
                po4 = oh_pool.tile([P, 4, 512], BF16, name="oh_sb")
                for oc in range(4):
                    nc.vector.tensor_scalar_add(
                        out=po4[:, oc, :], in0=acc_ap(oc),
                        scalar1=bias_sb[:, oc:oc + 1])
                nc.sync.dma_start(out=out.ap()[qm], in_=po4)

            # ---- projection machinery (chunked by 512 tokens) ----
            with (
                tc.tile_pool(name="xT_pool", bufs=1) as xT_pool,
                tc.tile_pool(name="wa_pool", bufs=1) as wa_pool,
            ):
                # input loads, priority-ordered: the V projection needs
                # wv + token chunk 0 (2MB) -> first PE work at ~5.6us;
                # wqk (split across both DMA queues) lands next so the
                # QK projection starts right as the V projection drains.
                xT_sb = xT_pool.tile([P, 4, ND, 512], BF16)
                wv_sb = wa_pool.tile([P, ND, 512], BF16)
                wqk_sb = wa_pool.tile([P, 2, ND, 512], BF16)
                # Staged loads: wave 1 = wv + xT chunk 0 (the V
                # projection's inputs) alone on two queues; the wqk
                # halves are HELD by tiny WAR-gate copies that read xT
                # chunk 0, so their transfers only start once wave 1
                # lands and never steal its bandwidth. xT chunk 1 and wp
                # are gated one wave later the same way (the gate copies
                # are emitted after the prelude so they don't block the
                # vector queue's PSUM evacuations).
                # wave 1a = first contraction halves (1MB) so the
                # half-V-projection starts as early as possible
                nc.sync.dma_start(out=xT_sb[:, 0, 0:4, :],
                                  in_=xT.ap()[0][:, 0:4, :])
                nc.scalar.dma_start(out=wv_sb[:, 0:4, :],
                                    in_=wv.ap()[:, 0:4, :])
                nc.sync.dma_start(out=xT_sb[:, 0, 4:8, :],
                                  in_=xT.ap()[0][:, 4:8, :])
                nc.scalar.dma_start(out=wv_sb[:, 4:8, :],
                                    in_=wv.ap()[:, 4:8, :])
                nc.sync.dma_start(out=wqk_sb[:, 0, 0:4, :],
                                  in_=wqk.ap()[:, 0, 0:4, :])
                nc.scalar.dma_start(out=wqk_sb[:, 0, 4:8, :],
                                    in_=wqk.ap()[:, 0, 4:8, :])
                nc.sync.dma_start(out=wqk_sb[:, 1, 0:4, :],
                                  in_=wqk.ap()[:, 1, 0:4, :])
                nc.scalar.dma_start(out=wqk_sb[:, 1, 4:8, :],
                                    in_=wqk.ap()[:, 1, 4:8, :])
                nc.vector.memset(ones_sb, 1.0)
                nc.vector.memset(ones_row, 1.0)
                nc.vector.tensor_copy(out=v1[:, :, :, DH:DH + 1], in_=ones_sb)
                make_upper_triangular(nc, tri, val=1.0, diag=True)

                def qk_group(pt, qc):
                    ps = ps_mm.tile([P, 512], F32, name="ps_qk", tag="ps_mm")
                    for dd in range(ND):
                        nc.tensor.matmul(
                            ps,
                            lhsT=wqk_sb[:, pt // 4, dd,
                                        (pt % 4) * P:(pt % 4 + 1) * P],
                            rhs=xT_sb[:, qc, dd, :],
                            start=(dd == 0), stop=(dd == ND - 1))
                    nc.vector.tensor_copy(
                        out=qkT[:, pt, qc * 512:(qc + 1) * 512], in_=ps)

                def v_mms(tt, ps, dds):
                    for dd in dds:
                        nc.tensor.matmul(
                            ps,
                            lhsT=xT_sb[:, tt // 4, dd,
                                       (tt % 4) * P:(tt % 4 + 1) * P],
                            rhs=wv_sb[:, dd, :],
                            start=(dd == 0), stop=(dd == ND - 1))

                def v_evac(tt, ps):
                    nc.vector.tensor_copy(
                        out=v1[:, tt, :, 0:DH],
                        in_=ps.rearrange("p (h d) -> p h d", h=HL))

                def v_group(tt):
                    ps = ps_mm.tile([P, 512], F32, name="ps_v", tag="ps_mm")
                    v_mms(tt, ps, range(ND))
                    v_evac(tt, ps)

                def v_chunk0_split():
                    # contraction-halved pairs: the dd 0-3 matmuls need
                    # only the first 1MB of input; dd 4-7 follow as the
                    # second wave lands (2 PSUM tiles live at a time)
                    for ta, tb in ((0, 1), (2, 3)):
                        psa = ps_mm.tile([P, 512], F32, name="ps_v",
                                         tag="ps_mm")
                        psb = ps_mm.tile([P, 512], F32, name="ps_v",
                                         tag="ps_mm")
                        v_mms(ta, psa, range(0, 4))
                        v_mms(tb, psb, range(0, 4))
                        v_mms(ta, psa, range(4, 8))
                        v_mms(tb, psb, range(4, 8))
                        v_evac(ta, psa)
                        v_evac(tb, psb)

                def proj_chunk_groups(qc):
                    for tt in range(4 * qc, 4 * qc + 4):
                        yield lambda tt=tt: v_group(tt)
                    for pt in range(8):
                        yield lambda pt=pt: qk_group(pt, qc)

                # prelude: project chunk 0 (V first, contraction-halved so
                # it starts after the first 1MB of input)
                v_chunk0_split()
                for pt in range(8):
                    qk_group(pt, 0)

                # remaining input loads: queue order IS the release
                # order (each queue drains serially), no gate copies
                for qc in range(1, 4):
                    nc.gpsimd.dma_start(out=xT_sb[:, qc], in_=xT.ap()[qc])
                nc.scalar.dma_start(out=wp2_sb, in_=wp.ap())
                nc.scalar.dma_start(out=bias_sb, in_=bias.ap())

                # q-macros 0..2: weave the next chunk's projections and
                # the PREVIOUS q-macro's out-projection into each one
                from itertools import chain as _chain
                og_prev = None
                for qm in range(3):
                    weave = proj_chunk_groups(qm + 1)
                    if og_prev is not None:
                        weave = _chain(weave,
                                       out_proj_groups(qm - 1, og_prev))
                    og_prev = attention(qm, weave)
                    for thunk in weave:  # drain leftovers
                        thunk()

            # q-macro 3: weave the first half of qm2's out-projection into
            # its pairs; the rest runs AFTER the attention so the PE has
            # real work (and stays at full clock) while the last half-
            # gather is in flight.
            og2_list = list(out_proj_groups(2, og_prev))
            weave3 = iter(og2_list)
            parts3 = attention(3, weave3)
            for thunk in weave3:
                thunk()
            final_out_proj(3, parts3)

    nc.finalize()
    return nc


_NC_CACHE = None


def _get_nc():
    global _NC_CACHE
    if _NC_CACHE is None:
        _NC_CACHE = build_bass()
    return _NC_CACHE


def _make_in_maps(x, w_qkv, w_proj, b_proj):
    x = np.asarray(x, np.float32)
    w_qkv = np.asarray(w_qkv, np.float32)
    w_proj = np.asarray(w_proj, np.float32)
    b_proj = np.asarray(b_proj, np.float32)
    wq, wk, wv_full = w_qkv[:, :D], w_qkv[:, D:2 * D], w_qkv[:, 2 * D:]
    scale = DH ** -0.5
    in_maps = []
    def pm(w, *shape):
        """(D, M) row-major -> partition-major SBUF layout, contiguous."""
        return np.ascontiguousarray(
            w.reshape(*shape).transpose(1, 0, 2)).astype(NP_BF16)

    for c in range(8):
        b, g = c // 2, c % 2
        cols = slice(g * 512, (g + 1) * 512)
        wqk_c = np.stack([(wq[:, cols] * scale).reshape(ND, P, 512),
                         wk[:, cols].reshape(ND, P, 512)],
                        axis=0).transpose(2, 0, 1, 3)
        # xT[qc, p, n, t'] = x[b][qc*512+t', n*128+p]
        xT_c = np.ascontiguousarray(
            x[b].T.reshape(ND, P, 4, 512).transpose(2, 1, 0, 3)
        ).astype(NP_BF16)
        # wp2[p, r, d, c'] = w_proj[r*512 + d*128 + p, cols][c']
        wp_c = np.ascontiguousarray(
            w_proj[:, cols].reshape(2, 4, P, 512).transpose(2, 0, 1, 3)
        ).astype(NP_BF16)
        in_maps.append({
            "xT": xT_c,
            "wqk": np.ascontiguousarray(wqk_c).astype(NP_BF16),
            "wv": pm(wv_full[:, cols], ND, P, 512),
            "wp": wp_c,
            "bias": np.ascontiguousarray(b_proj[cols].reshape(4, P).T),
        })
    return in_maps


def _assemble(results):
    out = np.empty((B, T, D), np.float32)
    for c in range(8):
        b, g = c // 2, c % 2
        res = results[c]["out"].astype(np.float32)
        out[b, :, g * 512:(g + 1) * 512] = (
            res.transpose(0, 3, 2, 1).reshape(T, 512))
    return out


def kernel(x, w_qkv, w_proj, b_proj):
    nc = _get_nc()
    in_maps = _make_in_maps(x, w_qkv, w_proj, b_proj)
    res = run_bass_kernel_spmd(nc, in_maps, core_ids=list(range(8)))
    return _assemble(res.results)


def kernel_traced(x, w_qkv, w_proj, b_proj, **kw):
    """Like kernel() but returns (output, BassKernelResults) with trace."""
    nc = _get_nc()
    in_maps = _make_in_maps(x, w_qkv, w_proj, b_proj)
    res = run_bass_kernel_spmd(nc, in_maps, core_ids=list(range(8)),
                               trace=True, **kw)
    return _assemble(res.results), res
